# revision 1
# baseline (speedup 1.0000x reference)
"""ENAS controller sampler, data-parallel over B=512 rollouts on 8 NeuronCores.

Strategy (per sharding_hint): shard the batch of independent rollouts across
the 8 cores (64 rollouts/core), params replicated. The per-rollout program is
the exact reference computation (38 sequential LSTM steps + attention-based
Gumbel top-k parent sampling + Gumbel argmax op sampling), compiled once via
PJRT for the neuron devices and executed with pmap. Matmul precision is pinned
to fp32 ("highest") because the discrete sampling decisions flip under bf16.

Falls back to CPU execution if no neuron devices are usable, so the function
always returns correct full-shape outputs.
"""
import os

os.environ.setdefault("NEURON_CC_FLAGS", "")
if "--auto-cast" not in os.environ.get("NEURON_CC_FLAGS", ""):
    os.environ["NEURON_CC_FLAGS"] = (
        os.environ.get("NEURON_CC_FLAGS", "") + " --auto-cast=none"
    ).strip()

import jax
import jax.numpy as jnp
import numpy as np

H = 2048
N = 14
P = 2
O = 8
B = 512
T = 5.0
TC = 2.5
OTR = 2.5
EPS = 1e-10
NDEV = 8


def _gumbel(u):
    return -jnp.log(-jnp.log(u + EPS) + EPS)


def _sample_one(u_p, u_o, emb, w_ih, b_ih, w_hh, b_hh, w_soft, b_soft,
                b_soft_no_learn, w_attn_1, w_attn_2, v_attn):
    def lstm(x, h, c):
        g = (w_ih @ x + b_ih + w_hh @ h + b_hh).reshape(4, H)
        i, f, gg, o = (jax.nn.sigmoid(g[0]), jax.nn.sigmoid(g[1]),
                       jnp.tanh(g[2]), jax.nn.sigmoid(g[3]))
        c = f * c + i * gg
        return o * jnp.tanh(c), c

    h = jnp.zeros(H, jnp.float32)
    c = jnp.zeros(H, jnp.float32)
    anchors = jnp.zeros((N, H), jnp.float32)
    anchors_w1 = jnp.zeros((N, H), jnp.float32)
    x = emb[0]
    for lid in range(2):
        h, c = lstm(x, h, c)
        anchors_w1 = anchors_w1.at[lid].set(w_attn_1 @ h)

    adj = jnp.zeros((N, N), jnp.float32).at[0, 1].set(1.0)
    ops = jnp.zeros(N - 2, jnp.int32)
    log_prob = jnp.float32(0.0)
    entropy = jnp.float32(0.0)

    for lid in range(2, N):
        h, c = lstm(x, h, c)
        q = jnp.tanh(anchors_w1[:lid] + w_attn_2 @ h)
        logits = TC * jnp.tanh((q @ v_attn) / T)
        logp = jax.nn.log_softmax(logits)
        idx = jax.lax.top_k(logits + _gumbel(u_p[lid - 2, :lid]), P)[1]
        adj = adj.at[idx, lid].set(1.0)
        log_prob = log_prob + jnp.mean(-logp[idx])
        entropy = entropy - jax.lax.stop_gradient(jnp.sum(jnp.exp(logp) * logp))
        x = jnp.take(anchors, idx[-1], axis=0)

        h, c = lstm(x, h, c)
        logits = (TC / OTR) * jnp.tanh((w_soft @ h + b_soft) / T) + b_soft_no_learn
        logp = jax.nn.log_softmax(logits)
        op = jnp.argmax(logits + _gumbel(u_o[lid - 2]))
        ops = ops.at[lid - 2].set(op.astype(jnp.int32))
        log_prob = log_prob - logp[op]
        entropy = entropy - jax.lax.stop_gradient(jnp.sum(jnp.exp(logp) * logp))

        x = jnp.take(emb, op + 1, axis=0)
        h, c = lstm(x, h, c)
        anchors = anchors.at[lid].set(h)
        anchors_w1 = anchors_w1.at[lid].set(w_attn_1 @ h)
        x = emb[0]

    return adj, ops, log_prob, entropy


def _batch(u_parent, u_op, *params):
    f = lambda up, uo: _sample_one(up, uo, *params)
    return jax.vmap(f)(u_parent, u_op)


_PARAM_NAMES = ("emb", "w_ih", "b_ih", "w_hh", "b_hh", "w_soft", "b_soft",
                "b_soft_no_learn", "w_attn_1", "w_attn_2", "v_attn")


def _run_sharded(inputs, devices):
    """Data-parallel pmap over the rollout batch; params replicated."""
    ndev = len(devices)
    per = B // ndev
    up = inputs["u_parent"].reshape(ndev, per, N - 2, N)
    uo = inputs["u_op"].reshape(ndev, per, N - 2, O)
    params = tuple(inputs[k] for k in _PARAM_NAMES)
    in_axes = (0, 0) + (None,) * len(params)
    with jax.default_matmul_precision("highest"):
        pf = jax.pmap(_batch, in_axes=in_axes, devices=devices)
        adj, ops, lp, ent = pf(up, uo, *params)
    adj = np.asarray(adj).reshape(B, N, N).astype(np.float32)
    ops = np.asarray(ops).reshape(B, N - 2).astype(np.int32)
    lp = np.asarray(lp).reshape(B).astype(np.float32)
    ent = np.asarray(ent).reshape(B).astype(np.float32)
    return adj, ops, lp, ent


def kernel(**inputs):
    inputs = {k: np.asarray(v) for k, v in inputs.items()}
    try:
        devices = [d for d in jax.devices() if d.platform != "cpu"][:NDEV]
        if len(devices) == NDEV:
            return _run_sharded(inputs, devices)
    except Exception:
        pass
    # Fallback: exact computation on CPU (still correct, not accelerated).
    cpu = jax.devices("cpu")
    with jax.default_matmul_precision("highest"):
        return _run_sharded(inputs, cpu[:1])
